# revision 1
# baseline (speedup 1.0000x reference)
"""CoxNAM Trainium2 kernel.

Computation (per feature f, for each batch row b):
    h1 = relu(x[b,f] * W1[f] + b1[f])        # [H1=256]
    h2 = relu(h1 @ W2[f] + b2[f])            # [H2=128]
    out[b] = sum_f (h2 @ W3[f] + b3[f])      # scalar

Sharding: features F=256 split across 8 NeuronCores (32 each, SPMD — one
program, per-core input shards). Per-core partials are summed on the host
along with sum(b3).

Per-core dataflow (h on partitions, batch on the free axis):
  A (PE):  z1[h,b] = W1[f,h]*x[b,f] + b1[f,h] as K=2 matmuls (ones row
           carries the bias), 4 features packed into the 4 PE row-groups
           concurrently via tile_position.
  B (DVE/ACT): h1 = relu(z1), bias-free, so feature-pairs are fused into
           one [128,1024] PSUM->SBUF op (two adjacent PSUM banks).
  C (PE):  z2[k,b] = sum_h W2[f,h,k] h1[h,b]  (K=256 in 2 chunks, accum)
  D (DVE/ACT): t = relu(z2 + b2)    PSUM -> SBUF bf16
  E (PE):  acc[32j, q*1024+bt*512+b] += sum_k W3[f,k] t[k,b], M=1 matmuls
           col-packed 4-wide, accumulated in PSUM over all features; one
           full-height copy + strided DMA drains the 4 rows per quarter.
"""

import os

import numpy as np
import ml_dtypes

F, B, H1, H2 = 256, 4096, 256, 128
NCORES = 8
BT = 512  # batch-tile width (one PSUM bank of fp32)
HC = H1 // 128  # h-chunks per feature
JW = 4  # feature packing width (PE row/col groups)
BTH = 2  # batch tiles per outer round (E-accumulator width = BTH*BT)

_CACHE = {}


def _jax_cache_setup():
    import jax

    d = os.path.join(os.path.expanduser("~"), ".cache", "coxnam_jaxcache")
    os.makedirs(d, exist_ok=True)
    jax.config.update("jax_compilation_cache_dir", d)
    jax.config.update("jax_persistent_cache_min_compile_time_secs", 0.0)
    jax.config.update("jax_persistent_cache_min_entry_size_bytes", 0)


def build_nc(fl=F // NCORES, b=B, dtype_name="bf16"):
    """Build the SPMD Bass program for one core holding `fl` features."""
    from contextlib import ExitStack

    import concourse.mybir as mybir
    import concourse.tile as tile
    from concourse import bacc

    dt = mybir.dt
    sdt = dt.bfloat16 if dtype_name == "bf16" else dt.float32
    f32r = dtype_name == "f32r"
    nbt = b // BT
    nq = nbt // BTH  # outer rounds
    ng = fl // JW  # feature groups of 4
    assert fl % JW == 0 and nbt % BTH == 0
    W2B = BTH * BT  # E-accumulator / drain width

    nc = bacc.Bacc("TRN2", target_bir_lowering=False, debug=False)
    # full 128-partition images: rows 32j = x_f / W1_f, rows 32j+1 = ones/b1_f
    xgi = nc.dram_tensor("xgi", [ng * 128, b], sdt, kind="ExternalInput").ap()
    w1gi = nc.dram_tensor("w1gi", [ng * 128, H1], sdt, kind="ExternalInput").ap()
    w2r = nc.dram_tensor("w2r", [128, fl * HC * H2], sdt, kind="ExternalInput").ap()
    b2t = nc.dram_tensor("b2t", [H2, fl], dt.float32, kind="ExternalInput").ap()
    w3 = nc.dram_tensor("w3", [H2, fl], sdt, kind="ExternalInput").ap()
    out = nc.dram_tensor("out", [JW, b], dt.float32, kind="ExternalOutput").ap()

    Relu = mybir.ActivationFunctionType.Relu
    add_, max_ = mybir.AluOpType.add, mybir.AluOpType.max

    def mm(ap):
        return ap.bitcast(dt.float32r) if f32r else ap

    # greedy DVE/ACT balancing for the PSUM-read epilogues
    ns = {"v": 0.0, "s": 0.0}

    def balanced(kind, out_ap, in_ap, bias_ap, width):
        tv = (120 + width) / 0.96
        ts = (172 + width) / 1.2
        use_v = ns["v"] + tv <= ns["s"] + ts
        if use_v:
            ns["v"] += tv
        else:
            ns["s"] += ts
        if kind == "relu":
            if use_v:
                nc.vector.tensor_scalar_max(out_ap, in_ap, 0.0)
            else:
                nc.scalar.activation(out_ap, in_ap, Relu)
        elif kind == "bias_relu":
            if use_v:
                nc.vector.tensor_scalar(out_ap, in_ap, bias_ap, 0.0, op0=add_, op1=max_)
            else:
                nc.scalar.activation(out_ap, in_ap, Relu, bias=bias_ap)
        else:  # copy
            if use_v:
                nc.vector.tensor_copy(out_ap, in_ap)
            else:
                nc.scalar.copy(out_ap, in_ap)

    with tile.TileContext(nc) as tc, ExitStack() as ctx:
        const = ctx.enter_context(tc.tile_pool(name="const", bufs=1))
        # xg[g]: feature 4g+j's x row at partition 32j, ones at 32j+1
        xg = [const.tile([128, b], sdt, name=f"xg{g}") for g in range(ng)]
        w1g = [const.tile([128, H1], sdt, name=f"w1g{g}") for g in range(ng)]
        w2s = const.tile([128, fl * HC * H2], sdt, name="w2s")
        b2s = const.tile([H2, fl], dt.float32, name="b2s")
        w3s = const.tile([H2, fl], sdt, name="w3s")

        nc.sync.dma_start(b2s[:], b2t[:])
        nc.sync.dma_start(w3s[:], w3[:])
        w2chunk = JW * HC * H2
        # quarter-split the x image and f-split g0's W2 chunk so the first
        # A/C matmuls start as soon as their slice lands, not after 1MB+
        qb = b // 4
        for g in range(ng):
            nc.sync.dma_start(
                w1g[g][:], w1gi[g * 128 : (g + 1) * 128, :]
            )
            for qq in range(4):
                nc.sync.dma_start(
                    xg[g][:, qq * qb : (qq + 1) * qb],
                    xgi[g * 128 : (g + 1) * 128, qq * qb : (qq + 1) * qb],
                )
            sub = w2chunk // JW
            for ff in range(JW if g == 0 else 1):
                w = sub if g == 0 else w2chunk
                nc.sync.dma_start(
                    w2s[:, g * w2chunk + ff * sub : g * w2chunk + ff * sub + w],
                    w2r[:, g * w2chunk + ff * sub : g * w2chunk + ff * sub + w],
                )

        pa = ctx.enter_context(tc.tile_pool(name="pa", bufs=2, space="PSUM"))
        pc = ctx.enter_context(tc.tile_pool(name="pc", bufs=2, space="PSUM"))
        pe = ctx.enter_context(tc.tile_pool(name="pe", bufs=1, space="PSUM"))
        hp = ctx.enter_context(tc.tile_pool(name="hp", bufs=14, space="SBUF"))
        tp = ctx.enter_context(tc.tile_pool(name="tp", bufs=4, space="SBUF"))

        for q in range(nq):
            pes = pe.tile([128, W2B], dt.float32, tag="pes", name=f"pes{q}")
            # full-height drain below reads rows the E-matmuls never write
            nc.vector.memset(pes[:], 0.0)
            for g in range(ng):
                hts = {}
                for bt in range(BTH):
                    babs = q * BTH + bt
                    bs = slice(babs * BT, (babs + 1) * BT)
                    for hc in range(HC):
                        za2 = [
                            pa.tile([128, 2 * BT], dt.float32, tag="za", name=f"za{p}")
                            for p in range(2)
                        ]
                        for j in range(JW):
                            p, i = divmod(j, 2)
                            nc.tensor.matmul(
                                za2[p][:, i * BT : (i + 1) * BT],
                                mm(w1g[g][32 * j : 32 * j + 2, hc * 128 : hc * 128 + 128]),
                                mm(xg[g][32 * j : 32 * j + 2, bs]),
                                start=True,
                                stop=True,
                                tile_position=(32 * j, 0),
                            )
                        for p in range(2):
                            ht = hp.tile([128, 2 * BT], sdt, tag="ht", name=f"ht{p}")
                            balanced("relu", ht[:], za2[p][:], None, 2 * BT)
                            hts[p, hc, bt] = ht
                for j in range(JW):
                    f = JW * g + j
                    p, i = divmod(j, 2)
                    for bt in range(BTH):
                        zc = pc.tile([H2, BT], dt.float32, tag="zc", name="zc")
                        for hc in range(HC):
                            nc.tensor.matmul(
                                zc[:],
                                mm(
                                    w2s[
                                        :,
                                        (f * HC + hc) * H2 : (f * HC + hc + 1) * H2,
                                    ]
                                ),
                                mm(hts[p, hc, bt][:, i * BT : (i + 1) * BT]),
                                start=(hc == 0),
                                stop=(hc == HC - 1),
                            )
                        tt = tp.tile([H2, BT], sdt, tag="tt", name="tt")
                        balanced("bias_relu", tt[:], zc[:], b2s[:, f : f + 1], BT)
                        nc.tensor.matmul(
                            pes[32 * j : 32 * j + 1, bt * BT : (bt + 1) * BT],
                            mm(w3s[:, f : f + 1]),
                            mm(tt[:]),
                            start=(g == 0),
                            stop=(g == ng - 1),
                            tile_position=(0, 32 * j),
                        )
            ot = tp.tile([128, W2B], dt.float32, tag="ot", name="ot")
            balanced("copy", ot[:], pes[:], None, W2B)
            nc.sync.dma_start(out[:, q * W2B : (q + 1) * W2B], ot[0:128:32, :])

    nc.compile()
    return nc


def make_in_maps(x, W1, b1, W2, b2, W3, ncores=NCORES, dtype_name="bf16"):
    """Host-side shard + layout prep. Inputs are np.float32 full tensors."""
    fl = F // ncores
    npdt = ml_dtypes.bfloat16 if dtype_name == "bf16" else np.float32
    W1f = W1.reshape(F, H1)
    W3f = W3.reshape(F, H2)

    def cast(a):
        return np.ascontiguousarray(a).astype(npdt)

    in_maps = []
    for c in range(ncores):
        fs = slice(c * fl, (c + 1) * fl)
        ng = fl // JW
        xgi = np.zeros((ng * 128, x.shape[0]), dtype=npdt)
        xgi[0::32] = cast(x[:, fs].T)
        xgi[1::32] = npdt(1.0)
        w1gi = np.zeros((ng * 128, H1), dtype=npdt)
        w1gi[0::32] = cast(W1f[fs])
        w1gi[1::32] = cast(b1[fs])
        # w2r[p, (f*HC+hc)*H2+k] = W2[f, hc*128+p, k]
        w2r_c = (
            W2[fs]
            .reshape(fl, HC, 128, H2)
            .transpose(2, 0, 1, 3)
            .reshape(128, fl * HC * H2)
        )
        in_maps.append(
            {
                "xgi": xgi,
                "w1gi": w1gi,
                "w2r": cast(w2r_c),
                "b2t": np.ascontiguousarray(b2[fs].T, dtype=np.float32),
                "w3": cast(W3f[fs].T),
            }
        )
    return in_maps


def kernel(x, W1, b1, W2, b2, W3, b3, _trace=False):
    _jax_cache_setup()
    from concourse.bass_utils import run_bass_kernel_spmd

    x = np.asarray(x, dtype=np.float32)
    W1 = np.asarray(W1, dtype=np.float32)
    b1 = np.asarray(b1, dtype=np.float32)
    W2 = np.asarray(W2, dtype=np.float32)
    b2 = np.asarray(b2, dtype=np.float32)
    W3 = np.asarray(W3, dtype=np.float32)
    b3 = np.asarray(b3, dtype=np.float32)

    if "nc" not in _CACHE:
        _CACHE["nc"] = build_nc()
    nc = _CACHE["nc"]

    in_maps = make_in_maps(x, W1, b1, W2, b2, W3)
    res = run_bass_kernel_spmd(nc, in_maps, core_ids=list(range(NCORES)), trace=_trace)
    total = np.zeros(B, dtype=np.float64)
    for c in range(NCORES):
        total += res.results[c]["out"].astype(np.float64).sum(axis=0)
    total += float(b3.sum())
    outv = total.astype(np.float32)[:, None]
    if _trace:
        kernel.last_results = res
    return outv

